# revision 30
# baseline (speedup 1.0000x reference)
"""ContinuousTimeRNN Trainium2 kernel (v3: PSUM-resident h, fp16 matmuls).

Data-parallel over batch N=512 across 8 NeuronCores (NS=64 rows each).
h is kept transposed (H on partitions, [128, 4, 64]) and lives in PSUM.
Per step one PSUM accumulation group computes

  h' = (I) @ hist + sum_k W_rec[k].T @ a[k] + [W_in;b].T @ x3

where hist holds 0.9*h in fp16 (y compensated by W_out/0.9 on the host),
the identity matmul OPENS the group (start=True sets the PSUM has_written
bits), and all moving tensors are fp16 (1 cycle/row at any PE p-state).
ACT reads PSUM first (tanh -> fp16; Tile serializes same-PSUM readers in
emission order, so tanh must precede the DVE hist copy), then DVE
produces a = 0.1*relu(tanh) and the hist slot; 16 W_rec chunk matmuls +
4 x matmuls accumulate behind the identity opener.
y: one 4-matmul PSUM quarter per step interleaved into PE idle slots.
"""

import sys

sys.path.insert(0, "/opt/trn_rl_repo")

import numpy as np

ALPHA = 0.1
T, N, H, DIN, DOUT, INIT = 1000, 512, 512, 2, 2, 2
NCORES = 8
NS = N // NCORES          # 64 batch rows per core
WIN = 25                  # y-flush window (steps)
BODY = 2 * WIN            # steps per For_i body
NK = H // 128             # 4 H-chunks
QW = WIN * NS // 4        # 400 columns per y quarter


def _build_nc(t_total=T, reps=1, hwloop=True, tanh_first=True, fillers=0):
    import concourse.mybir as mybir
    from concourse import bacc
    from concourse.tile import TileContext
    from concourse.masks import make_identity
    from concourse.bass import ds

    fp32 = mybir.dt.float32
    fp16 = mybir.dt.float16
    AFT = mybir.ActivationFunctionType

    nc = bacc.Bacc("TRN2", target_bir_lowering=False, debug=False,
                   num_devices=NCORES)

    # -------- DRAM I/O (per core) --------
    wrec_d = nc.dram_tensor("wrec", [NK, 128, H], fp16, kind="ExternalInput").ap()
    win3_d = nc.dram_tensor("win3", [DIN + 1, H], fp16, kind="ExternalInput").ap()
    wout_d = nc.dram_tensor("wout", [NK, 128, DOUT], fp16, kind="ExternalInput").ap()
    fcw3_d = nc.dram_tensor("fcw3", [INIT + 1, H], fp32, kind="ExternalInput").ap()
    init3_d = nc.dram_tensor("init3", [INIT + 1, NS], fp32, kind="ExternalInput").ap()
    xt_d = nc.dram_tensor("xt", [DIN + 1, t_total * NS], fp16, kind="ExternalInput").ap()
    # yA holds rows with (t % 50) < 25; yB holds the rest, shifted +BODY rows
    ya_d = nc.dram_tensor("ya", [DOUT, t_total * NS], fp32, kind="ExternalOutput").ap()
    yb_d = nc.dram_tensor("yb", [DOUT, (t_total + BODY) * NS], fp32,
                          kind="ExternalOutput").ap()

    with TileContext(nc) as tc:
        with (
            tc.tile_pool(name="wpool", bufs=1) as wpool,
            tc.tile_pool(name="hpool", bufs=1) as hpool,
            tc.tile_pool(name="apool", bufs=4) as apool,
            tc.tile_pool(name="xpool", bufs=2) as xpool,
            tc.tile_pool(name="ypool", bufs=2) as ypool,
            tc.tile_pool(name="hps", bufs=1, space="PSUM") as hpspool,
            tc.tile_pool(name="py", bufs=2, space="PSUM") as pypool,
        ):
            # -------- persistent SBUF --------
            wrec_sb = wpool.tile([128, NK, H], fp16)       # W_rec chunks (0.1 in a)
            win3_sb = wpool.tile([DIN + 1, H], fp16)       # 0.1*[W_in; bias]
            wout_sb = wpool.tile([128, NK, DOUT], fp16)    # W_out/0.9 chunks
            fcw3_sb = wpool.tile([INIT + 1, H], fp32)      # [fc_w.T; fc_b]
            init3_sb = wpool.tile([INIT + 1, NS], fp32)    # [initdir.T; ones]
            ident = wpool.tile([128, 128], fp16)           # identity (fp16)
            # h history ring (fp16, holds 0.9*h_after): slot s-1 feeds the
            # identity matmul of step s and the y matmuls (W_out/0.9)
            hist = hpool.tile([128, NK, BODY * NS], fp16)

            for k in range(NK):
                nc.sync.dma_start(out=wrec_sb[:, k, :], in_=wrec_d[k])
                nc.sync.dma_start(out=wout_sb[:, k, :], in_=wout_d[k])
            nc.sync.dma_start(out=win3_sb[:], in_=win3_d)
            nc.sync.dma_start(out=fcw3_sb[:], in_=fcw3_d)
            nc.sync.dma_start(out=init3_sb[:], in_=init3_d)
            make_identity(nc, ident[:])

            # Two independent 32-row batch streams per core, pipelined half
            # a step apart: per-stream ping-pong PSUM tiles; h0 -> tile 1 of
            # each stream (prev of s=0).  NS2 = NS // 2 rows per stream.
            NS2 = NS // 2
            hps = {}
            for st in range(2):
                for pp in range(2):
                    hps_t = hpspool.tile([128, NK, NS2], fp32,
                                         tag=f"hps{st}{pp}")
                    hps[(st, pp)] = hps_t
            for st in range(2):
                for m in range(NK):
                    nc.tensor.matmul(
                        hps[(st, 1)][:, m, :],
                        fcw3_sb[:, m * 128:(m + 1) * 128],
                        init3_sb[:, st * NS2:(st + 1) * NS2],
                        start=True, stop=True)

            # -------- time loop --------
            def body_ivs():
                if hwloop:
                    with tc.For_i(0, reps, 1) as _rep, \
                         tc.For_i(0, t_total, BODY) as iv:
                        yield iv
                else:
                    yield from range(0, t_total, BODY)

            for iv in body_ivs():
                xbuf = xpool.tile([DIN + 1, BODY * NS], fp16)
                nc.sync.dma_start(out=xbuf[:], in_=xt_d[:, ds(iv * NS, BODY * NS)])

                # y quarters pending per step (4 matmuls + DVE copy + DMA):
                # s=1..4   : prev-body window B (hist slots 25..49) -> yB
                # s=26..29 : this-body window A (hist slots 0..24)  -> yA
                for s in range(BODY):
                    pp, qq = (0, 1) if s % 2 == 0 else (1, 0)
                    slot = (s - 1) % BODY

                    for st in range(2):
                        cur, prev = hps[(st, pp)], hps[(st, qq)]
                        c0 = slot * NS + st * NS2   # hist cols of this stream
                        tbuf = apool.tile([128, NK * NS2], fp16,
                                          tag=f"tbuf{st}")
                        abuf = apool.tile([128, NK, NS2], fp16,
                                          tag=f"abuf{st}")

                        # tanh then relu(0.1*x)=0.1*relu(x) both on ACT (no
                        # ACT->DVE->PE double hop before aW); the DVE hist
                        # copy runs concurrently with ACT's relu and feeds
                        # the next step's identity opener earlier
                        nc.scalar.activation(tbuf[:], prev[:].rearrange(
                            "p k n -> p (k n)"), AFT.Tanh)
                        nc.scalar.activation(
                            abuf[:].rearrange("p k n -> p (k n)"), tbuf[:],
                            AFT.Relu, scale=ALPHA)
                        nc.vector.tensor_scalar_mul(
                            hist[:, :, c0:c0 + NS2], prev[:], 1.0 - ALPHA)

                        # h' accumulation group: identity opener + 16 aW + 4 x
                        nc.tensor.matmul(
                            cur[:],
                            ident[:],
                            hist[:, :, c0:c0 + NS2],
                            start=True, stop=False, skip_group_check=True)
                        for k in range(NK):
                            for m in range(NK):
                                nc.tensor.matmul(
                                    cur[:, m, :],
                                    wrec_sb[:, k, m * 128:(m + 1) * 128],
                                    abuf[:, k, :],
                                    start=False, stop=False,
                                    skip_group_check=True)
                        for m in range(NK):
                            nc.tensor.matmul(
                                cur[:, m, :],
                                win3_sb[:, m * 128:(m + 1) * 128],
                                xbuf[:, s * NS + st * NS2:
                                     s * NS + (st + 1) * NS2],
                                start=False, stop=True, skip_group_check=True)

                    # p-state fillers: keep PE streaming through the
                    # activation wait so the 2.4 GHz ramp engages
                    for _f in range(fillers):
                        nc.tensor.matmul(scratch[:], wrec_sb[:, 0, 0:128],
                                         wrec_sb[:, 0, :],
                                         start=True, stop=True,
                                         skip_group_check=True)

                    # interleaved y work, spread ONE matmul per step so a
                    # 4-matmul burst never lands between the stream groups
                    yk = None
                    if 1 <= s <= 16:       # prev-body window B -> yB (+BODY)
                        q, kk = divmod(s - 1, 4)
                        yk = (WIN * NS, q, kk, yb_d,
                              iv * NS + WIN * NS + q * QW)
                    elif 26 <= s <= 41:    # this-body window A -> yA
                        q, kk = divmod(s - 26, 4)
                        yk = (0, q, kk, ya_d, iv * NS + q * QW)
                    if yk is not None:
                        colbase, q, kk, ydst, yoff = yk
                        if kk == 0:
                            ypy = pypool.tile([DOUT, QW], fp32)
                            cur_py = ypy
                        nc.tensor.matmul(
                            cur_py[:], wout_sb[:, kk, :],
                            hist[:, kk, colbase + q * QW:
                                 colbase + (q + 1) * QW],
                            start=(kk == 0), stop=(kk == 3))
                        if kk == 3:
                            ysb = ypool.tile([DOUT, QW], fp32, tag="ysb")
                            nc.vector.tensor_copy(ysb[:], cur_py[:])
                            nc.sync.dma_start(out=ydst[:, ds(yoff, QW)],
                                              in_=ysb[:])

            # -------- post-loop: final window B (rows T-25..T-1) --------
            fpp = (BODY - 1) % 2
            for st in range(2):
                nc.vector.tensor_scalar_mul(
                    hist[:, :, (BODY - 1) * NS + st * NS2:
                         (BODY - 1) * NS + (st + 1) * NS2],
                    hps[(st, fpp)][:], 1.0 - ALPHA)
            for q in range(4):
                py = pypool.tile([DOUT, QW], fp32)
                ysb = ypool.tile([DOUT, QW], fp32, tag="ysb")
                for k in range(NK):
                    nc.tensor.matmul(
                        py[:], wout_sb[:, k, :],
                        hist[:, k, WIN * NS + q * QW: WIN * NS + (q + 1) * QW],
                        start=(k == 0), stop=(k == NK - 1))
                nc.vector.tensor_copy(ysb[:], py[:])
                nc.sync.dma_start(
                    out=yb_d[:, (t_total + WIN) * NS + q * QW:
                             (t_total + WIN) * NS + (q + 1) * QW],
                    in_=ysb[:])

    nc.compile()
    return nc


_NC_CACHE = {}


def _get_nc():
    if "nc" not in _NC_CACHE:
        _NC_CACHE["nc"] = _build_nc()
    return _NC_CACHE["nc"]


def _prep_in_maps(initdir, velocities, fc_w, fc_b, W_in, W_rec, W_out, bias):
    initdir = np.asarray(initdir, np.float32)
    velocities = np.asarray(velocities, np.float32)
    fc_w = np.asarray(fc_w, np.float32)
    fc_b = np.asarray(fc_b, np.float32)
    W_in = np.asarray(W_in, np.float32)
    W_rec = np.asarray(W_rec, np.float32)
    W_out = np.asarray(W_out, np.float32)
    bias = np.asarray(bias, np.float32)

    # host-side weight prep (shared across cores)
    wrec = W_rec.reshape(NK, 128, H).astype(np.float16)
    win3 = (ALPHA * np.concatenate([W_in, bias[None, :]], axis=0)).astype(np.float16)
    wout = (W_out / (1.0 - ALPHA)).reshape(NK, 128, DOUT).astype(np.float16)
    fcw3 = np.concatenate([fc_w.T, fc_b[None, :]], axis=0)           # (3, H)

    in_maps = []
    for c in range(NCORES):
        sl = slice(c * NS, (c + 1) * NS)
        init3 = np.concatenate([initdir[sl].T,
                                np.ones((1, NS), np.float32)], axis=0)
        # xt[p, t*NS+n] = velocities[t, c*NS+n, p]; row DIN = ones
        xs = velocities[:, sl, :]                                    # (T, NS, 2)
        xt = np.empty((DIN + 1, T * NS), np.float32)
        xt[:DIN] = xs.transpose(2, 0, 1).reshape(DIN, T * NS)
        xt[DIN] = 1.0
        in_maps.append({
            "wrec": np.ascontiguousarray(wrec),
            "win3": np.ascontiguousarray(win3),
            "wout": np.ascontiguousarray(wout),
            "fcw3": np.ascontiguousarray(fcw3),
            "init3": np.ascontiguousarray(init3),
            "xt": xt.astype(np.float16),
        })
    return in_maps


def _unpack(res):
    out = np.empty((T, N, DOUT), np.float32)
    tmask = (np.arange(T) % BODY) < WIN
    for c in range(NCORES):
        ya = res.results[c]["ya"].reshape(DOUT, T, NS)
        yb = res.results[c]["yb"].reshape(DOUT, T + BODY, NS)
        yt = np.where(tmask[None, :, None], ya, yb[:, BODY:, :])
        out[:, c * NS:(c + 1) * NS, :] = yt.transpose(1, 2, 0)
    return out


def kernel(initdir, velocities, fc_w, fc_b, W_in, W_rec, W_out, bias):
    from concourse.bass_utils import run_bass_kernel_spmd

    in_maps = _prep_in_maps(initdir, velocities, fc_w, fc_b, W_in, W_rec,
                            W_out, bias)
    nc = _get_nc()
    res = run_bass_kernel_spmd(nc, in_maps, list(range(NCORES)))
    return _unpack(res)
